# revision 40
# baseline (speedup 1.0000x reference)
"""Causal self-attention (B=2, T=2048, C=1024, H=16) on 8 trn2 NeuronCores.

Sharding: (batch, head-group). Core c owns batch c//4 and heads
[4*(c%4) .. 4*(c%4)+3] (4 heads = 256 features). The 4 partial c_proj
outputs per batch are summed on the host ("all-reduce after c_proj"),
plus the effective bias.

Precision/layout scheme (validated rel_err ~4e-3 vs the 2e-2 gate):
  - Q,K projections: fp8e4 DoubleRow matmuls (x fp8 + w fp8). On TRN2
    DoubleRow doubles the CONTRACTION DEPTH per streamed column (256
    rows vs 128), so the 1024-deep projection needs 4 accumulation
    matmuls instead of 8 -> real 2x. w_q is NOT pre-scaled by
    1/sqrt(d) (that underflows fp8e4 normals); the softmax scale rides
    on the exp activation's `scale` operand instead.
  - V projection, S, P, PV, c_proj: bf16 (fp8 V / w_proj arithmetic
    breaks the error budget; fp8 P buys nothing since the PE streams
    one 128-row column per cycle regardless of moving dtype, and fp8
    activation output measurably slows the ACT engine).
  - S: the two heads of a pair run concurrently on PE row groups
    0:64 / 64:128. Causal masking: pre-zeroed diagonal P tiles + one
    strided tri-mask multiply per diagonal block on DVE.
  - Softmax denominators ride as a 65th ones-column in V; reciprocals
    via the batched fast-approx DVE op.
  - DMAs are consolidated into few multi-dim transfers (DRAM tensors
    pre-arranged host-side to SBUF element order) because each
    dma_start costs ~700ns of issue time on its engine - many small
    DMAs stall the prologue on issue rate, not bandwidth.
Bias folding (host): K bias drops (softmax row-shift invariance), V
bias folds into b_proj as b_v @ w_proj. Only the Q bias stays in-kernel.
"""

import sys

sys.path.insert(0, "/opt/trn_rl_repo")

import numpy as np
import ml_dtypes

BF16 = ml_dtypes.bfloat16
F8 = ml_dtypes.float8_e4m3fn

N_CORES = 8
B, T, C = 2, 2048, 1024
H, D = 16, 64
HPC = 4                       # heads per core
NPAIR = 2                     # head pairs per core
F = HPC * D                   # local feature width = 256
TT = T                        # tokens per core = 2048
TCH = 512                     # token chunk (moving-operand width)
NCH = TT // TCH               # 4 token chunks
KB = 128                      # kv block size
NBLK = TT // KB               # 16 kv blocks

_COMPILED = {}


def _build():
    import concourse.bass as bass
    import concourse.mybir as mybir
    import concourse.tile as tile
    from concourse import bacc

    f32, bf16 = mybir.dt.float32, mybir.dt.bfloat16
    f8 = mybir.dt.float8e4
    Exp = mybir.ActivationFunctionType.Exp
    DR = mybir.MatmulPerfMode.DoubleRow

    nc = bacc.Bacc("TRN2", target_bir_lowering=False, debug=False,
                   num_devices=N_CORES)

    # DRAM layouts mirror the SBUF tiles (partition-major) so each logical
    # block moves with ONE dma_start.
    xt = nc.dram_tensor("xt", [128, C // 128, TT], bf16, kind="ExternalInput")
    x8 = nc.dram_tensor("x8", [128, C // 256, 2, TT], f8,
                        kind="ExternalInput")
    w8 = nc.dram_tensor("w8", [128, 4, 4, 2, 128], f8, kind="ExternalInput")
    wv = nc.dram_tensor("wv", [128, C // 128, 2 * 128], bf16,
                        kind="ExternalInput")
    bq2 = nc.dram_tensor("bq2", [128, 2], f32, kind="ExternalInput")
    wp = nc.dram_tensor("wp", [128, 2, C], bf16, kind="ExternalInput")
    # const blob: [tri2 (2*128) | eye (128) | ones (32)]
    cblob = nc.dram_tensor("cblob", [128, 2 * KB + 128 + 32], bf16,
                           kind="ExternalInput")
    out = nc.dram_tensor("out", [TT, C], bf16, kind="ExternalOutput")

    with tile.TileContext(nc) as tc, \
         nc.allow_low_precision(reason="fp8/bf16 matmul pipeline, fp32 psum"):
        with tc.tile_pool(name="const", bufs=1) as cpool, \
             tc.tile_pool(name="seq", bufs=1) as seq, \
             tc.tile_pool(name="work", bufs=4) as work, \
             tc.tile_pool(name="psBig", bufs=2, space="PSUM") as psBig, \
             tc.tile_pool(name="psS", bufs=2, space="PSUM") as psS, \
             tc.tile_pool(name="psY", bufs=2, space="PSUM") as psY:

            # ---- weights first (idle queues) so the first matmuls only
            #      wait on x arrival. Few LARGE transfers: the PE p-state
            #      rewards dense late bursts over early dribbles. The tiny
            #      Q-bias rides right behind w8 (a late bias stalls
            #      attention; bq2 ahead of w8 costs ~2us of fixed latency).
            w8_sb = cpool.tile([128, 4, 4, 2, 128], f8)
            nc.scalar.dma_start(w8_sb[:], w8[:])
            x8_sb = cpool.tile([128, 4, 2, TT], f8)
            nc.sync.dma_start(x8_sb[:, :, :, 0:TCH], x8[:, :, :, 0:TCH])
            b_sb = cpool.tile([128, 2], f32)
            nc.scalar.dma_start(b_sb[:], bq2[:])
            cb_sb = cpool.tile([128, 2 * KB + 128 + 32], bf16)
            nc.scalar.dma_start(cb_sb[:], cblob[:])
            tri_sb = cb_sb[:, 0:2 * KB].rearrange("p (a q) -> p a q", a=2)
            eye_sb = cb_sb[:, 2 * KB:2 * KB + 128]
            ones_sb = cb_sb[:, 2 * KB + 128:]
            wv_sb = cpool.tile([128, 8, 2 * 128], bf16)
            nc.scalar.dma_start(wv_sb[:], wv[:])

            xt_sb = cpool.tile([128, C // 128, TT], bf16)
            nc.gpsimd.dma_start(xt_sb[:, :, 0:TCH], xt[:, :, 0:TCH])
            wp_sb = cpool.tile([128, 2, C], bf16)
            nc.scalar.dma_start(wp_sb[:], wp[:])
            for t in range(1, NCH):
                tsl = slice(t * TCH, (t + 1) * TCH)
                nc.sync.dma_start(x8_sb[:, :, :, tsl], x8[:, :, :, tsl])
                nc.gpsimd.dma_start(xt_sb[:, :, tsl], xt[:, :, tsl])

            # ---- resident sequence tensors, per (pair, 512-token chunk) ----
            qt_t = [[seq.tile([128, TCH], bf16, tag=f"qt{p}{t}",
                              name=f"qt{p}{t}") for t in range(NCH)]
                    for p in range(NPAIR)]
            kt_t = [[seq.tile([128, TCH], bf16, tag=f"kt{p}{t}",
                              name=f"kt{p}{t}") for t in range(NCH)]
                    for p in range(NPAIR)]
            yt_t = [[seq.tile([128, TCH], bf16, tag=f"yt{p}{t}",
                              name=f"yt{p}{t}") for t in range(NCH)]
                    for p in range(NPAIR)]
            # v65[p][:, i, 0:65] = [V_head0 | 1], [:, i, 65:130] = [V_head1 | 1]
            v65 = []
            for p in range(NPAIR):
                v = seq.tile([128, NBLK, 130], bf16, name=f"v65{p}")
                nc.vector.tensor_copy(
                    v[:, :, 64::65],
                    ones_sb[:, :].rearrange("p (a b) -> p a b", b=2))
                v65.append(v)

            # ---- pre-zeroed diagonal P tiles ([128, 1024], halves=heads) ----
            p_diag = {}
            for r in (1, 2, 3):
                for j in (0, 1):
                    pt = seq.tile([128, 2 * TCH], bf16, name=f"pdiag{r}_{j}")
                    pt3 = pt[:].rearrange("p (a q) -> p a q", a=2)
                    nc.vector.memset(pt3[:, :, 0:128 * r].bitcast(f32), 0.0)
                    p_diag[(r, j)] = pt

            # per-(bq,pair) denominator tiles [1, 2*TCH] fp32 (halves=heads)
            den_tiles = {}

            def qkv_qk_gen(t):
                """Q,K projection for one 512-token chunk (fp8 DoubleRow,
                256-deep slabs)."""
                tsl = slice(t * TCH, (t + 1) * TCH)
                for s in range(4):
                    part, pair = s // 2, s % 2
                    ps = psBig.tile([128, TCH], f32, tag="big",
                                    name=f"pqk{t}_{s}")
                    for sl in range(4):
                        nc.tensor.matmul(
                            ps[:], w8_sb[:, sl, s], x8_sb[:, sl, :, tsl],
                            start=(sl == 0), stop=(sl == 3), perf_mode=DR)
                        if sl == 1:
                            yield
                    if part == 0:
                        # only Q keeps a bias (K's drops under softmax
                        # shift-invariance, V's folds into b_proj); t0 on
                        # the then-idle ACT engine
                        if t == 0:
                            nc.scalar.add(qt_t[pair][t][:], ps[:],
                                          b_sb[:, s:s + 1])
                        else:
                            nc.vector.tensor_scalar_add(qt_t[pair][t][:],
                                                        ps[:],
                                                        b_sb[:, s:s + 1])
                    else:
                        nc.vector.tensor_copy(kt_t[pair][t][:], ps[:])
                    yield

            def qkv_v_gen(t):
                """V projection + transpose into v65 for one chunk."""
                tsl = slice(t * TCH, (t + 1) * TCH)
                # V: bf16, 128-deep blocks
                vt_tmp = [None, None]
                for pair in range(NPAIR):
                    ps = psBig.tile([128, TCH], f32, tag="big",
                                    name=f"pv{t}_{pair}")
                    for cb in range(8):
                        nc.tensor.matmul(
                            ps[:], wv_sb[:, cb, pair * 128:(pair + 1) * 128],
                            xt_sb[:, cb, tsl],
                            start=(cb == 0), stop=(cb == 7))
                        if cb == 3:
                            yield
                    vt_tmp[pair] = work.tile([128, TCH], bf16, tag="vt",
                                             bufs=2, name=f"vt{t}_{pair}")
                    nc.vector.tensor_copy(vt_tmp[pair][:], ps[:])
                    yield
                for pair in range(NPAIR):
                    ptr = psS.tile([128, TCH], bf16, tag="s",
                                   name=f"ptr{t}_{pair}")
                    for i in range(4):
                        nc.tensor.transpose(ptr[:, i * 128:(i + 1) * 128],
                                            vt_tmp[pair][:, i * 128:(i + 1) * 128],
                                            eye_sb[:])
                        if i == 1:
                            yield
                    # single strided copy: [kv, blk, head, d] <- [kv, blk*d]
                    t4 = t * 4
                    nc.vector.tensor_copy(
                        v65[pair][:, t4:t4 + 4, :].rearrange(
                            "p a (h c) -> p a h c", c=65)[:, :, :, 0:64],
                        ptr[:].rearrange("p (a h c) -> p a h c", a=4, h=2))
                    yield

            def norm_pair_gen(bq, pair, fused_ps=None):
                """Softmax normalization for one head pair of a chunk.
                With fused_ps (tail path) the psum->sbuf copy and the
                normalize multiply collapse into one scalar_tensor_tensor,
                shortening the critical tail chain."""
                rec = work.tile([1, 2 * TCH], f32, tag="rec", bufs=4,
                                name=f"rec{bq}{pair}")
                if fused_ps is not None:
                    # tail path: per-head pipeline, reciprocal -> PE ones-row
                    # broadcast (PE is idle; gpsimd serializes) -> fused
                    # psum-normalize-copy
                    for hh in range(2):
                        hs = hh * 64
                        hsl = slice(hh * TCH, (hh + 1) * TCH)
                        nc.vector.reciprocal_approx_fast(
                            rec[:, hsl], den_tiles[(bq, pair)][:, hsl])
                        bcast = work.tile([128, TCH], f32, tag="bcast",
                                          bufs=4, name=f"bcf{pair}{hh}")
                        nc.gpsimd.partition_broadcast(bcast[:], rec[:, hsl])
                        nc.vector.scalar_tensor_tensor(
                            yt_t[pair][bq][hs:hs + 64, :],
                            fused_ps[hh][0:64, :], 1.0,
                            bcast[0:64, :],
                            op0=mybir.AluOpType.mult,
                            op1=mybir.AluOpType.mult)
                        yield
                    return
                nc.vector.reciprocal_approx_fast(rec[:],
                                                 den_tiles[(bq, pair)][:])
                yield
                for hh in range(2):
                    hs = hh * 64
                    bcast = work.tile([128, TCH], f32, tag="bcast", bufs=4,
                                      name=f"bcast{bq}{pair}{hh}")
                    nc.gpsimd.partition_broadcast(
                        bcast[:], rec[:, hh * TCH:(hh + 1) * TCH])
                    nc.vector.tensor_mul(yt_t[pair][bq][hs:hs + 64, :],
                                         yt_t[pair][bq][hs:hs + 64, :],
                                         bcast[hs:hs + 64, :])
                    yield

            def proj_gen(bq, tail=False):
                """c_proj for one 512-token chunk. At the tail, psS is free
                (exp done) - borrow it so six psum regions are in flight and
                the psum->sbuf copies leave the matmul critical path."""
                ps_wide = {}
                if tail:
                    for ic in (0, 1):
                        ps_wide[ic] = psS.tile([128, 2 * TCH], f32, tag="s",
                                               name=f"pjw{ic}")
                for ic in range(4):
                    tt_i = bq * 4 + ic
                    for cc in range(2):
                        if ic in ps_wide:
                            pj = ps_wide[ic][:, cc * TCH:(cc + 1) * TCH]
                        else:
                            pj = psBig.tile([128, TCH], f32, tag="big",
                                            name=f"pj{tt_i}_{cc}")[:]
                        for pair in range(NPAIR):
                            nc.tensor.matmul(
                                pj,
                                yt_t[pair][bq][:, ic * 128:(ic + 1) * 128],
                                wp_sb[:, pair, cc * TCH:(cc + 1) * TCH],
                                start=(pair == 0), stop=(pair == 1))
                        ost = work.tile([128, TCH], bf16, tag="ost", bufs=8,
                                        name=f"ost{tt_i}_{cc}")
                        if tail and (ic + cc) % 2 == 0:
                            nc.scalar.copy(ost[:], pj)
                        else:
                            nc.vector.tensor_copy(ost[:], pj)
                        # steady-state stores ride the sync queue; at the
                        # tail the queues split the last 1MB with gpsimd
                        # first (so its end-of-program drain overlaps proj)
                        # and scalar last (after its ost copies)
                        if tail:
                            qs = (nc.gpsimd, nc.gpsimd, nc.sync, nc.sync,
                                  nc.sync, nc.sync, nc.scalar, nc.scalar)
                            eng = qs[ic * 2 + cc]
                        else:
                            eng = nc.sync
                        eng.dma_start(
                            out[tt_i * 128:(tt_i + 1) * 128,
                                cc * TCH:(cc + 1) * TCH], ost[:])
                        yield

            class Filler:
                """Paces background generators. Tile deps only attach to
                already-emitted producers, so a consumer must never be
                emitted before its producer: drain_through(k) force-emits
                every generator keyed <= k before its consumers go out."""

                def __init__(self):
                    self.gens = []

                def add(self, g, key=None):
                    self.gens.append((key, g))

                def step(self):
                    while self.gens:
                        try:
                            next(self.gens[0][1])
                            return
                        except StopIteration:
                            self.gens.pop(0)

                def drain_through(self, key):
                    while self.gens and self.gens[0][0] is not None \
                            and self.gens[0][0] <= key:
                        k, g = self.gens.pop(0)
                        for _ in g:
                            pass

                def drain(self):
                    while self.gens:
                        for _ in self.gens.pop(0)[1]:
                            pass

            def attn_pair(pair, bq, bk, use_idx):
                """S for a head pair into one [128,1024] psum tile + one exp.
                The two heads' S matmuls run concurrently on PE row groups
                0:64 / 64:128. exp applies the 1/sqrt(d) softmax scale."""
                kchunk = bk // 4
                kcol = (bk % 4) * 128
                s_ps = psS.tile([128, 2 * TCH], f32, tag="s",
                                name=f"s{pair}{bq}{bk}")
                r = bk - 4 * bq
                trim = 128 * r if r > 0 else 0
                for hh in range(2):
                    hs = hh * 64
                    nc.tensor.matmul(
                        s_ps[:, hh * TCH + trim:(hh + 1) * TCH],
                        kt_t[pair][kchunk][hs:hs + 64, kcol:kcol + 128],
                        qt_t[pair][bq][hs:hs + 64, trim:],
                        start=True, stop=True)
                if r < 0:
                    p_t = work.tile([128, 2 * TCH], bf16, tag="p", bufs=6,
                                    name=f"p{pair}{bq}{bk}")
                    nc.scalar.activation(p_t[:], s_ps[:], Exp, scale=0.125)
                    return p_t
                if r == 0:
                    p_t = work.tile([128, 2 * TCH], bf16, tag="p", bufs=6,
                                    name=f"p{pair}{bq}{bk}")
                    nc.scalar.activation(p_t[:], s_ps[:], Exp, scale=0.125)
                else:
                    p_t = p_diag[(r, use_idx % 2)]
                    s3 = s_ps[:].rearrange("p (a q) -> p a q", a=2)
                    p3 = p_t[:].rearrange("p (a q) -> p a q", a=2)
                    nc.scalar.activation(p3[:, :, 128 * r:],
                                         s3[:, :, 128 * r:], Exp, scale=0.125)
                # one strided mask multiply covering both heads
                p3m = p_t[:].rearrange("p (a q) -> p a q", a=2)
                nc.vector.tensor_mul(p3m[:, :, 128 * r:128 * r + 128],
                                     p3m[:, :, 128 * r:128 * r + 128],
                                     tri_sb[:])
                return p_t

            def attention_pair_chunk(pair, bq, fl, fuse=False, pre=None):
                nsteps = 1
                nblk = 4 * bq + 4
                yt_ps = [psY.tile([65, TCH], f32, tag="yt",
                                  name=f"ytps{pair}{bq}{hh}")
                         for hh in range(2)]
                pend = None

                def emit_pv(bk, p_t, stop):
                    r = bk - 4 * bq
                    trim = 128 * r if r > 0 else 0
                    for hh in range(2):
                        nc.tensor.matmul(
                            yt_ps[hh][:, trim:],
                            v65[pair][:, bk, 65 * hh:65 * hh + 65],
                            p_t[:, hh * TCH + trim:(hh + 1) * TCH],
                            start=(bk == 0), stop=stop)

                for bk in range(nblk):
                    if pre is not None:
                        p_t = pre[(pair, bk)]
                    else:
                        p_t = attn_pair(pair, bq, bk, bq + pair)
                    if pend is not None:
                        emit_pv(pend[0], pend[1], stop=False)
                    pend = (bk, p_t)
                    for _ in range(nsteps):
                        fl.step()
                emit_pv(pend[0], pend[1], stop=True)
                den = work.tile([1, 2 * TCH], f32, tag="den", bufs=10,
                                name=f"den{bq}{pair}")
                for hh in range(2):
                    hs = hh * 64
                    # den on ACT, yt on DVE: the two psum readers run in
                    # parallel so the psY buffer frees sooner (the next
                    # pair's first PV waits on it)
                    nc.scalar.copy(den[:, hh * TCH:(hh + 1) * TCH],
                                   yt_ps[hh][64:65, :])
                    if not fuse:
                        nc.vector.tensor_copy(yt_t[pair][bq][hs:hs + 64, :],
                                              yt_ps[hh][0:64, :])
                den_tiles[(bq, pair)] = den
                fl.step()
                return yt_ps

            def chain(*gens):
                for g in gens:
                    yield from g

            # ---- schedule ----
            # chunk-0 Q,K first; then bq0's S+exp (they only need q,k and
            # run while xt/V still stream in); V emits after so the PE
            # in-order queue isn't blocked by transposes waiting on xt
            for _ in qkv_qk_gen(0):
                pass
            pre0 = {}
            for pair in range(NPAIR):
                for bk in range(4):
                    pre0[(pair, bk)] = attn_pair(pair, 0, bk, pair)
            for _ in qkv_v_gen(0):
                pass
            fl = Filler()
            for t in range(1, NCH):
                fl.add(chain(qkv_qk_gen(t), qkv_v_gen(t)), key=t)
            pending_np = []
            last = NCH - 1
            for bq in range(NCH):
                fl.drain_through(bq)
                if bq == last:
                    # make all pending norm+proj work available as filler
                    # so only the final chunk's chain remains at the tail
                    while pending_np:
                        fl.add(pending_np.pop(0))
                for pair in range(NPAIR):
                    fuse = (bq == last and pair == NPAIR - 1)
                    yt_ps = attention_pair_chunk(
                        pair, bq, fl, fuse=fuse,
                        pre=pre0 if bq == 0 else None)
                    if bq == last:
                        # normalize each pair while the next one (or the
                        # tail) is still streaming
                        fl.add(norm_pair_gen(bq, pair,
                                             fused_ps=yt_ps if fuse
                                             else None))
                if bq != last:
                    if pending_np:
                        fl.add(pending_np.pop(0))
                    pending_np.append(chain(norm_pair_gen(bq, 0),
                                            norm_pair_gen(bq, 1),
                                            proj_gen(bq)))
                else:
                    fl.add(proj_gen(bq, tail=True))
            fl.drain()
    nc.compile()
    return nc


def _get_nc():
    if "nc" not in _COMPILED:
        _COMPILED["nc"] = _build()
    return _COMPILED["nc"]


def _prep_in_maps(x, w_attn, b_attn, w_proj):
    x = np.asarray(x, np.float32)
    w_attn = np.asarray(w_attn, np.float32)
    b_attn = np.asarray(b_attn, np.float32)
    w_proj = np.asarray(w_proj, np.float32)

    tri = np.triu(np.ones((KB, KB), np.float32))  # keep where q >= kv
    eye = np.eye(128, dtype=np.float32)
    ones = np.ones((128, 32), np.float32)
    cblob = np.ascontiguousarray(np.concatenate(
        [np.stack([tri, tri], axis=1).reshape(128, 2 * KB), eye, ones],
        axis=1)).astype(BF16)

    # xt[p, cb, t] = x^T[cb*128 + p, t]
    xts = [np.ascontiguousarray(
        x[b].T.reshape(8, 128, T).transpose(1, 0, 2)).astype(BF16)
        for b in range(B)]
    # x8[p, sl, i, t] = x^T[sl*256 + i*128 + p, t]
    x8s = [np.ascontiguousarray(
        x[b].T.reshape(4, 2, 128, T).transpose(2, 0, 1, 3)).astype(F8)
        for b in range(B)]

    in_maps = []
    for c in range(N_CORES):
        b, hg = c // 4, c % 4
        f0 = hg * F
        # w8[p, sl, s, i, cc] = w[sl*256 + i*128 + p, col(s, cc)], unscaled
        w8a = np.zeros((128, 4, 4, 2, 128), np.float32)
        slices_bq = []
        for s in range(4):
            part, pair = s // 2, s % 2
            lo = part * C + f0 + pair * 128
            wsl = w_attn[:, lo:lo + 128]          # [C, 128]
            if part == 0:
                slices_bq.append(b_attn[lo:lo + 128])
            w8a[:, :, s, :, :] = wsl.reshape(4, 2, 128, 128).transpose(
                2, 0, 1, 3)
        w8a = np.ascontiguousarray(w8a).astype(F8)
        bq2 = np.ascontiguousarray(np.stack(slices_bq, axis=1))
        # wv[p, cb, pair*128+cc] = w_v[cb*128 + p, f0 + pair*128 + cc]
        wva = np.ascontiguousarray(
            w_attn[:, 2 * C + f0:2 * C + f0 + F].reshape(
                8, 128, 256).transpose(1, 0, 2)).astype(BF16)
        # wp[p, pair, cc] = w_proj[f0 + pair*128 + p, cc]
        wpc = np.ascontiguousarray(
            w_proj[f0:f0 + F, :].reshape(2, 128, C).transpose(
                1, 0, 2)).astype(BF16)
        in_maps.append({
            "xt": xts[b], "x8": x8s[b], "w8": w8a, "wv": wva,
            "bq2": bq2, "wp": wpc, "cblob": cblob,
        })
    return in_maps


def _run(inputs, trace=False):
    from concourse.bass_utils import run_bass_kernel_spmd

    nc = _get_nc()
    in_maps = _prep_in_maps(inputs["x"], inputs["w_attn"], inputs["b_attn"],
                            inputs["w_proj"])
    res = run_bass_kernel_spmd(nc, in_maps, list(range(N_CORES)), trace=trace)
    b_attn = np.asarray(inputs["b_attn"], np.float32)
    w_proj = np.asarray(inputs["w_proj"], np.float32)
    # V bias passes through softmax (rows sum to 1) -> fold into b_proj
    b_eff = (np.asarray(inputs["b_proj"], np.float64)
             + b_attn[2 * C:].astype(np.float64) @ w_proj.astype(np.float64))
    y = np.zeros((B, TT, C), np.float32)
    for b in range(B):
        acc = np.zeros((TT, C), np.float64)
        for hg in range(4):
            acc += np.asarray(res.results[b * 4 + hg]["out"], np.float64)
        y[b] = (acc + b_eff).astype(np.float32)
    return y, res


def kernel(**inputs):
    y, _ = _run(inputs, trace=False)
    return y


# revision 41
# speedup vs baseline: 1.0045x; 1.0045x over previous
"""Causal self-attention (B=2, T=2048, C=1024, H=16) on 8 trn2 NeuronCores.

Sharding: (batch, head-group). Core c owns batch c//4 and heads
[4*(c%4) .. 4*(c%4)+3] (4 heads = 256 features). The 4 partial c_proj
outputs per batch are summed on the host ("all-reduce after c_proj"),
plus the effective bias.

Precision/layout scheme (validated rel_err ~4e-3 vs the 2e-2 gate):
  - Q,K projections: fp8e4 DoubleRow matmuls (x fp8 + w fp8). On TRN2
    DoubleRow doubles the CONTRACTION DEPTH per streamed column (256
    rows vs 128), so the 1024-deep projection needs 4 accumulation
    matmuls instead of 8 -> real 2x. w_q is NOT pre-scaled by
    1/sqrt(d) (that underflows fp8e4 normals); the softmax scale rides
    on the exp activation's `scale` operand instead.
  - V projection, S, P, PV, c_proj: bf16 (fp8 V / w_proj arithmetic
    breaks the error budget; fp8 P buys nothing since the PE streams
    one 128-row column per cycle regardless of moving dtype, and fp8
    activation output measurably slows the ACT engine).
  - S: the two heads of a pair run concurrently on PE row groups
    0:64 / 64:128. Causal masking: pre-zeroed diagonal P tiles + one
    strided tri-mask multiply per diagonal block on DVE.
  - Softmax denominators ride as a 65th ones-column in V; reciprocals
    via the batched fast-approx DVE op.
  - DMAs are consolidated into few multi-dim transfers (DRAM tensors
    pre-arranged host-side to SBUF element order) because each
    dma_start costs ~700ns of issue time on its engine - many small
    DMAs stall the prologue on issue rate, not bandwidth.
Bias folding (host): K bias drops (softmax row-shift invariance), V
bias folds into b_proj as b_v @ w_proj. Only the Q bias stays in-kernel.
"""

import sys

sys.path.insert(0, "/opt/trn_rl_repo")

import numpy as np
import ml_dtypes

BF16 = ml_dtypes.bfloat16
F8 = ml_dtypes.float8_e4m3fn

N_CORES = 8
B, T, C = 2, 2048, 1024
H, D = 16, 64
HPC = 4                       # heads per core
NPAIR = 2                     # head pairs per core
F = HPC * D                   # local feature width = 256
TT = T                        # tokens per core = 2048
TCH = 512                     # token chunk (moving-operand width)
NCH = TT // TCH               # 4 token chunks
KB = 128                      # kv block size
NBLK = TT // KB               # 16 kv blocks

_COMPILED = {}


def _build():
    import concourse.bass as bass
    import concourse.mybir as mybir
    import concourse.tile as tile
    from concourse import bacc

    f32, bf16 = mybir.dt.float32, mybir.dt.bfloat16
    f8 = mybir.dt.float8e4
    Exp = mybir.ActivationFunctionType.Exp
    DR = mybir.MatmulPerfMode.DoubleRow

    nc = bacc.Bacc("TRN2", target_bir_lowering=False, debug=False,
                   num_devices=N_CORES)

    # DRAM layouts mirror the SBUF tiles (partition-major) so each logical
    # block moves with ONE dma_start.
    xt = nc.dram_tensor("xt", [128, C // 128, TT], bf16, kind="ExternalInput")
    x8 = nc.dram_tensor("x8", [128, C // 256, 2, TT], f8,
                        kind="ExternalInput")
    w8 = nc.dram_tensor("w8", [128, 4, 4, 2, 128], f8, kind="ExternalInput")
    wv = nc.dram_tensor("wv", [128, C // 128, 2 * 128], bf16,
                        kind="ExternalInput")
    bq2 = nc.dram_tensor("bq2", [128, 2], f32, kind="ExternalInput")
    wp = nc.dram_tensor("wp", [128, 2, C], bf16, kind="ExternalInput")
    # const blob: [tri2 (2*128) | eye (128) | ones (32)]
    cblob = nc.dram_tensor("cblob", [128, 2 * KB + 128 + 32], bf16,
                           kind="ExternalInput")
    out = nc.dram_tensor("out", [TT, C], bf16, kind="ExternalOutput")

    with tile.TileContext(nc) as tc, \
         nc.allow_low_precision(reason="fp8/bf16 matmul pipeline, fp32 psum"):
        with tc.tile_pool(name="const", bufs=1) as cpool, \
             tc.tile_pool(name="seq", bufs=1) as seq, \
             tc.tile_pool(name="work", bufs=4) as work, \
             tc.tile_pool(name="psBig", bufs=2, space="PSUM") as psBig, \
             tc.tile_pool(name="psS", bufs=2, space="PSUM") as psS, \
             tc.tile_pool(name="psY", bufs=2, space="PSUM") as psY:

            # ---- weights first (idle queues) so the first matmuls only
            #      wait on x arrival. Few LARGE transfers: the PE p-state
            #      rewards dense late bursts over early dribbles. The tiny
            #      Q-bias rides right behind w8 (a late bias stalls
            #      attention; bq2 ahead of w8 costs ~2us of fixed latency).
            w8_sb = cpool.tile([128, 4, 4, 2, 128], f8)
            nc.scalar.dma_start(w8_sb[:], w8[:])
            x8_sb = cpool.tile([128, 4, 2, TT], f8)
            nc.sync.dma_start(x8_sb[:, :, :, 0:TCH], x8[:, :, :, 0:TCH])
            b_sb = cpool.tile([128, 2], f32)
            nc.scalar.dma_start(b_sb[:], bq2[:])
            cb_sb = cpool.tile([128, 2 * KB + 128 + 32], bf16)
            nc.scalar.dma_start(cb_sb[:], cblob[:])
            tri_sb = cb_sb[:, 0:2 * KB].rearrange("p (a q) -> p a q", a=2)
            eye_sb = cb_sb[:, 2 * KB:2 * KB + 128]
            ones_sb = cb_sb[:, 2 * KB + 128:]
            wv_sb = cpool.tile([128, 8, 2 * 128], bf16)
            nc.scalar.dma_start(wv_sb[:], wv[:])

            xt_sb = cpool.tile([128, C // 128, TT], bf16)
            nc.gpsimd.dma_start(xt_sb[:, :, 0:TCH], xt[:, :, 0:TCH])
            wp_sb = cpool.tile([128, 2, C], bf16)
            nc.scalar.dma_start(wp_sb[:], wp[:])
            for t in range(1, NCH):
                tsl = slice(t * TCH, (t + 1) * TCH)
                nc.sync.dma_start(x8_sb[:, :, :, tsl], x8[:, :, :, tsl])
                nc.gpsimd.dma_start(xt_sb[:, :, tsl], xt[:, :, tsl])

            # ---- resident sequence tensors, per (pair, 512-token chunk) ----
            qt_t = [[seq.tile([128, TCH], bf16, tag=f"qt{p}{t}",
                              name=f"qt{p}{t}") for t in range(NCH)]
                    for p in range(NPAIR)]
            kt_t = [[seq.tile([128, TCH], bf16, tag=f"kt{p}{t}",
                              name=f"kt{p}{t}") for t in range(NCH)]
                    for p in range(NPAIR)]
            yt_t = [[seq.tile([128, TCH], bf16, tag=f"yt{p}{t}",
                              name=f"yt{p}{t}") for t in range(NCH)]
                    for p in range(NPAIR)]
            # v65[p][:, i, 0:65] = [V_head0 | 1], [:, i, 65:130] = [V_head1 | 1]
            v65 = []
            for p in range(NPAIR):
                v = seq.tile([128, NBLK, 130], bf16, name=f"v65{p}")
                nc.vector.tensor_copy(
                    v[:, :, 64::65],
                    ones_sb[:, :].rearrange("p (a b) -> p a b", b=2))
                v65.append(v)

            # ---- pre-zeroed diagonal P tiles ([128, 1024], halves=heads) ----
            p_diag = {}
            for r in (1, 2, 3):
                for j in (0, 1):
                    pt = seq.tile([128, 2 * TCH], bf16, name=f"pdiag{r}_{j}")
                    pt3 = pt[:].rearrange("p (a q) -> p a q", a=2)
                    nc.vector.memset(pt3[:, :, 0:128 * r].bitcast(f32), 0.0)
                    p_diag[(r, j)] = pt

            # per-(bq,pair) denominator tiles [1, 2*TCH] fp32 (halves=heads)
            den_tiles = {}

            def qkv_qk_gen(t):
                """Q,K projection for one 512-token chunk (fp8 DoubleRow,
                256-deep slabs)."""
                tsl = slice(t * TCH, (t + 1) * TCH)
                for s in range(4):
                    part, pair = s // 2, s % 2
                    ps = psBig.tile([128, TCH], f32, tag="big",
                                    name=f"pqk{t}_{s}")
                    for sl in range(4):
                        nc.tensor.matmul(
                            ps[:], w8_sb[:, sl, s], x8_sb[:, sl, :, tsl],
                            start=(sl == 0), stop=(sl == 3), perf_mode=DR)
                        if sl == 1:
                            yield
                    if part == 0:
                        # only Q keeps a bias (K's drops under softmax
                        # shift-invariance, V's folds into b_proj); t0 on
                        # the then-idle ACT engine
                        if t == 0:
                            nc.scalar.add(qt_t[pair][t][:], ps[:],
                                          b_sb[:, s:s + 1])
                        else:
                            nc.vector.tensor_scalar_add(qt_t[pair][t][:],
                                                        ps[:],
                                                        b_sb[:, s:s + 1])
                    else:
                        nc.vector.tensor_copy(kt_t[pair][t][:], ps[:])
                    yield

            def qkv_v_gen(t):
                """V projection + transpose into v65 for one chunk."""
                tsl = slice(t * TCH, (t + 1) * TCH)
                # V: bf16, 128-deep blocks
                vt_tmp = [None, None]
                for pair in range(NPAIR):
                    ps = psBig.tile([128, TCH], f32, tag="big",
                                    name=f"pv{t}_{pair}")
                    for cb in range(8):
                        nc.tensor.matmul(
                            ps[:], wv_sb[:, cb, pair * 128:(pair + 1) * 128],
                            xt_sb[:, cb, tsl],
                            start=(cb == 0), stop=(cb == 7))
                        if cb == 3:
                            yield
                    vt_tmp[pair] = work.tile([128, TCH], bf16, tag="vt",
                                             bufs=2, name=f"vt{t}_{pair}")
                    nc.vector.tensor_copy(vt_tmp[pair][:], ps[:])
                    yield
                for pair in range(NPAIR):
                    ptr = psS.tile([128, TCH], bf16, tag="s",
                                   name=f"ptr{t}_{pair}")
                    for i in range(4):
                        nc.tensor.transpose(ptr[:, i * 128:(i + 1) * 128],
                                            vt_tmp[pair][:, i * 128:(i + 1) * 128],
                                            eye_sb[:])
                        if i == 1:
                            yield
                    # single strided copy: [kv, blk, head, d] <- [kv, blk*d]
                    t4 = t * 4
                    nc.vector.tensor_copy(
                        v65[pair][:, t4:t4 + 4, :].rearrange(
                            "p a (h c) -> p a h c", c=65)[:, :, :, 0:64],
                        ptr[:].rearrange("p (a h c) -> p a h c", a=4, h=2))
                    yield

            def norm_pair_gen(bq, pair, fused_ps=None):
                """Softmax normalization for one head pair of a chunk.
                With fused_ps (tail path) the psum->sbuf copy and the
                normalize multiply collapse into one scalar_tensor_tensor,
                shortening the critical tail chain."""
                rec = work.tile([1, 2 * TCH], f32, tag="rec", bufs=4,
                                name=f"rec{bq}{pair}")
                if fused_ps is not None:
                    # tail path: per-head pipeline, reciprocal -> PE ones-row
                    # broadcast (PE is idle; gpsimd serializes) -> fused
                    # psum-normalize-copy
                    for hh in range(2):
                        hs = hh * 64
                        hsl = slice(hh * TCH, (hh + 1) * TCH)
                        nc.vector.reciprocal_approx_fast(
                            rec[:, hsl], den_tiles[(bq, pair)][:, hsl])
                        bcast = work.tile([128, TCH], f32, tag="bcast",
                                          bufs=4, name=f"bcf{pair}{hh}")
                        nc.gpsimd.partition_broadcast(bcast[:], rec[:, hsl])
                        nc.vector.scalar_tensor_tensor(
                            yt_t[pair][bq][hs:hs + 64, :],
                            fused_ps[hh][0:64, :], 1.0,
                            bcast[0:64, :],
                            op0=mybir.AluOpType.mult,
                            op1=mybir.AluOpType.mult)
                        yield
                    return
                nc.vector.reciprocal_approx_fast(rec[:],
                                                 den_tiles[(bq, pair)][:])
                yield
                for hh in range(2):
                    hs = hh * 64
                    bcast = work.tile([128, TCH], f32, tag="bcast", bufs=4,
                                      name=f"bcast{bq}{pair}{hh}")
                    nc.gpsimd.partition_broadcast(
                        bcast[:], rec[:, hh * TCH:(hh + 1) * TCH])
                    nc.vector.tensor_mul(yt_t[pair][bq][hs:hs + 64, :],
                                         yt_t[pair][bq][hs:hs + 64, :],
                                         bcast[hs:hs + 64, :])
                    yield

            def proj_gen(bq, tail=False):
                """c_proj for one 512-token chunk. At the tail, psS is free
                (exp done) - borrow it so six psum regions are in flight and
                the psum->sbuf copies leave the matmul critical path."""
                ps_wide = {}
                if tail:
                    for ic in (0, 1):
                        ps_wide[ic] = psS.tile([128, 2 * TCH], f32, tag="s",
                                               name=f"pjw{ic}")
                for ic in range(4):
                    tt_i = bq * 4 + ic
                    for cc in range(2):
                        if ic in ps_wide:
                            pj = ps_wide[ic][:, cc * TCH:(cc + 1) * TCH]
                        else:
                            pj = psBig.tile([128, TCH], f32, tag="big",
                                            name=f"pj{tt_i}_{cc}")[:]
                        for pair in range(NPAIR):
                            nc.tensor.matmul(
                                pj,
                                yt_t[pair][bq][:, ic * 128:(ic + 1) * 128],
                                wp_sb[:, pair, cc * TCH:(cc + 1) * TCH],
                                start=(pair == 0), stop=(pair == 1))
                        ost = work.tile([128, TCH], bf16, tag="ost", bufs=8,
                                        name=f"ost{tt_i}_{cc}")
                        if tail and (ic + cc) % 2 == 0:
                            nc.scalar.copy(ost[:], pj)
                        else:
                            nc.vector.tensor_copy(ost[:], pj)
                        # steady-state stores ride the sync queue; at the
                        # tail the queues split the last 1MB with gpsimd
                        # first (so its end-of-program drain overlaps proj)
                        # and scalar last (after its ost copies)
                        if tail:
                            qs = (nc.gpsimd, nc.gpsimd, nc.sync, nc.sync,
                                  nc.sync, nc.sync, nc.scalar, nc.scalar)
                            eng = qs[ic * 2 + cc]
                        else:
                            eng = nc.sync
                        eng.dma_start(
                            out[tt_i * 128:(tt_i + 1) * 128,
                                cc * TCH:(cc + 1) * TCH], ost[:])
                        yield

            class Filler:
                """Paces background generators. Tile deps only attach to
                already-emitted producers, so a consumer must never be
                emitted before its producer: drain_through(k) force-emits
                every generator keyed <= k before its consumers go out."""

                def __init__(self):
                    self.gens = []

                def add(self, g, key=None):
                    self.gens.append((key, g))

                def step(self):
                    while self.gens:
                        try:
                            next(self.gens[0][1])
                            return
                        except StopIteration:
                            self.gens.pop(0)

                def drain_through(self, key):
                    while self.gens and self.gens[0][0] is not None \
                            and self.gens[0][0] <= key:
                        k, g = self.gens.pop(0)
                        for _ in g:
                            pass

                def drain(self):
                    while self.gens:
                        for _ in self.gens.pop(0)[1]:
                            pass

            def attn_pair(pair, bq, bk, use_idx):
                """S for a head pair into one [128,1024] psum tile + one exp.
                The two heads' S matmuls run concurrently on PE row groups
                0:64 / 64:128. exp applies the 1/sqrt(d) softmax scale."""
                kchunk = bk // 4
                kcol = (bk % 4) * 128
                s_ps = psS.tile([128, 2 * TCH], f32, tag="s",
                                name=f"s{pair}{bq}{bk}")
                r = bk - 4 * bq
                trim = 128 * r if r > 0 else 0
                for hh in range(2):
                    hs = hh * 64
                    nc.tensor.matmul(
                        s_ps[:, hh * TCH + trim:(hh + 1) * TCH],
                        kt_t[pair][kchunk][hs:hs + 64, kcol:kcol + 128],
                        qt_t[pair][bq][hs:hs + 64, trim:],
                        start=True, stop=True)
                if r < 0:
                    p_t = work.tile([128, 2 * TCH], bf16, tag="p", bufs=6,
                                    name=f"p{pair}{bq}{bk}")
                    nc.scalar.activation(p_t[:], s_ps[:], Exp, scale=0.125)
                    return p_t
                if r == 0:
                    p_t = work.tile([128, 2 * TCH], bf16, tag="p", bufs=6,
                                    name=f"p{pair}{bq}{bk}")
                    nc.scalar.activation(p_t[:], s_ps[:], Exp, scale=0.125)
                else:
                    p_t = p_diag[(r, use_idx % 2)]
                    s3 = s_ps[:].rearrange("p (a q) -> p a q", a=2)
                    p3 = p_t[:].rearrange("p (a q) -> p a q", a=2)
                    nc.scalar.activation(p3[:, :, 128 * r:],
                                         s3[:, :, 128 * r:], Exp, scale=0.125)
                # one strided mask multiply covering both heads
                p3m = p_t[:].rearrange("p (a q) -> p a q", a=2)
                nc.vector.tensor_mul(p3m[:, :, 128 * r:128 * r + 128],
                                     p3m[:, :, 128 * r:128 * r + 128],
                                     tri_sb[:])
                return p_t

            def attention_pair_chunk(pair, bq, fl, fuse=False, pre=None):
                nsteps = 1
                nblk = 4 * bq + 4
                yt_ps = [psY.tile([65, TCH], f32, tag="yt",
                                  name=f"ytps{pair}{bq}{hh}")
                         for hh in range(2)]
                pend = None

                def emit_pv(bk, p_t, stop):
                    r = bk - 4 * bq
                    trim = 128 * r if r > 0 else 0
                    for hh in range(2):
                        nc.tensor.matmul(
                            yt_ps[hh][:, trim:],
                            v65[pair][:, bk, 65 * hh:65 * hh + 65],
                            p_t[:, hh * TCH + trim:(hh + 1) * TCH],
                            start=(bk == 0), stop=stop)

                for bk in range(nblk):
                    if pre is not None:
                        p_t = pre[(pair, bk)]
                    else:
                        p_t = attn_pair(pair, bq, bk, bq + pair)
                    if pend is not None:
                        emit_pv(pend[0], pend[1], stop=False)
                    pend = (bk, p_t)
                    for _ in range(nsteps):
                        fl.step()
                emit_pv(pend[0], pend[1], stop=True)
                den = work.tile([1, 2 * TCH], f32, tag="den", bufs=10,
                                name=f"den{bq}{pair}")
                for hh in range(2):
                    hs = hh * 64
                    if fuse:
                        # tail: ACT is idle, keep DVE free for the recips
                        nc.scalar.copy(den[:, hh * TCH:(hh + 1) * TCH],
                                       yt_ps[hh][64:65, :])
                    else:
                        nc.vector.tensor_copy(
                            den[:, hh * TCH:(hh + 1) * TCH],
                            yt_ps[hh][64:65, :])
                        nc.vector.tensor_copy(yt_t[pair][bq][hs:hs + 64, :],
                                              yt_ps[hh][0:64, :])
                den_tiles[(bq, pair)] = den
                fl.step()
                return yt_ps

            def chain(*gens):
                for g in gens:
                    yield from g

            # ---- schedule ----
            # chunk-0 Q,K first; then bq0's S+exp (they only need q,k and
            # run while xt/V still stream in); V emits after so the PE
            # in-order queue isn't blocked by transposes waiting on xt
            for _ in qkv_qk_gen(0):
                pass
            pre0 = {}
            for pair in range(NPAIR):
                for bk in range(4):
                    pre0[(pair, bk)] = attn_pair(pair, 0, bk, pair)
            for _ in qkv_v_gen(0):
                pass
            fl = Filler()
            for t in range(1, NCH):
                fl.add(chain(qkv_qk_gen(t), qkv_v_gen(t)), key=t)
            pending_np = []
            last = NCH - 1
            for bq in range(NCH):
                fl.drain_through(bq)
                if bq == last:
                    # make all pending norm+proj work available as filler
                    # so only the final chunk's chain remains at the tail
                    while pending_np:
                        fl.add(pending_np.pop(0))
                for pair in range(NPAIR):
                    fuse = (bq == last and pair == NPAIR - 1)
                    yt_ps = attention_pair_chunk(
                        pair, bq, fl, fuse=fuse,
                        pre=pre0 if bq == 0 else None)
                    if bq == last:
                        # normalize each pair while the next one (or the
                        # tail) is still streaming
                        fl.add(norm_pair_gen(bq, pair,
                                             fused_ps=yt_ps if fuse
                                             else None))
                if bq != last:
                    if pending_np:
                        fl.add(pending_np.pop(0))
                    pending_np.append(chain(norm_pair_gen(bq, 0),
                                            norm_pair_gen(bq, 1),
                                            proj_gen(bq)))
                else:
                    fl.add(proj_gen(bq, tail=True))
            fl.drain()
    nc.compile()
    return nc


def _get_nc():
    if "nc" not in _COMPILED:
        _COMPILED["nc"] = _build()
    return _COMPILED["nc"]


def _prep_in_maps(x, w_attn, b_attn, w_proj):
    x = np.asarray(x, np.float32)
    w_attn = np.asarray(w_attn, np.float32)
    b_attn = np.asarray(b_attn, np.float32)
    w_proj = np.asarray(w_proj, np.float32)

    tri = np.triu(np.ones((KB, KB), np.float32))  # keep where q >= kv
    eye = np.eye(128, dtype=np.float32)
    ones = np.ones((128, 32), np.float32)
    cblob = np.ascontiguousarray(np.concatenate(
        [np.stack([tri, tri], axis=1).reshape(128, 2 * KB), eye, ones],
        axis=1)).astype(BF16)

    # xt[p, cb, t] = x^T[cb*128 + p, t]
    xts = [np.ascontiguousarray(
        x[b].T.reshape(8, 128, T).transpose(1, 0, 2)).astype(BF16)
        for b in range(B)]
    # x8[p, sl, i, t] = x^T[sl*256 + i*128 + p, t]
    x8s = [np.ascontiguousarray(
        x[b].T.reshape(4, 2, 128, T).transpose(2, 0, 1, 3)).astype(F8)
        for b in range(B)]

    in_maps = []
    for c in range(N_CORES):
        b, hg = c // 4, c % 4
        f0 = hg * F
        # w8[p, sl, s, i, cc] = w[sl*256 + i*128 + p, col(s, cc)], unscaled
        w8a = np.zeros((128, 4, 4, 2, 128), np.float32)
        slices_bq = []
        for s in range(4):
            part, pair = s // 2, s % 2
            lo = part * C + f0 + pair * 128
            wsl = w_attn[:, lo:lo + 128]          # [C, 128]
            if part == 0:
                slices_bq.append(b_attn[lo:lo + 128])
            w8a[:, :, s, :, :] = wsl.reshape(4, 2, 128, 128).transpose(
                2, 0, 1, 3)
        w8a = np.ascontiguousarray(w8a).astype(F8)
        bq2 = np.ascontiguousarray(np.stack(slices_bq, axis=1))
        # wv[p, cb, pair*128+cc] = w_v[cb*128 + p, f0 + pair*128 + cc]
        wva = np.ascontiguousarray(
            w_attn[:, 2 * C + f0:2 * C + f0 + F].reshape(
                8, 128, 256).transpose(1, 0, 2)).astype(BF16)
        # wp[p, pair, cc] = w_proj[f0 + pair*128 + p, cc]
        wpc = np.ascontiguousarray(
            w_proj[f0:f0 + F, :].reshape(2, 128, C).transpose(
                1, 0, 2)).astype(BF16)
        in_maps.append({
            "xt": xts[b], "x8": x8s[b], "w8": w8a, "wv": wva,
            "bq2": bq2, "wp": wpc, "cblob": cblob,
        })
    return in_maps


def _run(inputs, trace=False):
    from concourse.bass_utils import run_bass_kernel_spmd

    nc = _get_nc()
    in_maps = _prep_in_maps(inputs["x"], inputs["w_attn"], inputs["b_attn"],
                            inputs["w_proj"])
    res = run_bass_kernel_spmd(nc, in_maps, list(range(N_CORES)), trace=trace)
    b_attn = np.asarray(inputs["b_attn"], np.float32)
    w_proj = np.asarray(inputs["w_proj"], np.float32)
    # V bias passes through softmax (rows sum to 1) -> fold into b_proj
    b_eff = (np.asarray(inputs["b_proj"], np.float64)
             + b_attn[2 * C:].astype(np.float64) @ w_proj.astype(np.float64))
    y = np.zeros((B, TT, C), np.float32)
    for b in range(B):
        acc = np.zeros((TT, C), np.float64)
        for hg in range(4):
            acc += np.asarray(res.results[b * 4 + hg]["out"], np.float64)
        y[b] = (acc + b_eff).astype(np.float32)
    return y, res


def kernel(**inputs):
    y, _ = _run(inputs, trace=False)
    return y


# revision 42
# speedup vs baseline: 1.1846x; 1.1793x over previous
"""Causal self-attention (B=2, T=2048, C=1024, H=16) on 8 trn2 NeuronCores.

Sharding: (batch, head-group). Core c owns batch c//4 and heads
[4*(c%4) .. 4*(c%4)+3] (4 heads = 256 features). The 4 partial c_proj
outputs per batch are summed on the host ("all-reduce after c_proj"),
plus the effective bias.

Precision/layout scheme (measured rel_err ~1.1e-2 vs the 2e-2 gate):
  - Q,K projections: fp8e4 DoubleRow matmuls (x fp8 + w fp8). On TRN2
    DoubleRow doubles the CONTRACTION DEPTH per streamed column (256
    rows vs 128), so the 1024-deep projection needs 4 accumulation
    matmuls instead of 8 -> real 2x. w_q is NOT pre-scaled by
    1/sqrt(d) (that underflows fp8e4 normals); the softmax scale rides
    on the exp activation's `scale` operand instead.
  - V projection, S, P, PV, c_proj: bf16 (fp8 V / w_proj arithmetic
    breaks the error budget; fp8 P buys nothing since the PE streams
    one 128-row column per cycle regardless of moving dtype, and fp8
    activation output measurably slows the ACT engine).
  - S: the two heads of a pair run concurrently on PE row groups
    0:64 / 64:128. Causal masking: pre-zeroed diagonal P tiles + one
    strided tri-mask multiply per diagonal block on DVE.
  - Softmax denominators ride as a 65th ones-column in V; reciprocals
    via the batched fast-approx DVE op.
  - DMAs are consolidated into few multi-dim transfers (DRAM tensors
    pre-arranged host-side to SBUF element order) because each
    dma_start costs ~700ns of issue time on its engine - many small
    DMAs stall the prologue on issue rate, not bandwidth.
Bias folding (host): K bias drops (softmax row-shift invariance), V
bias folds into b_proj as b_v @ w_proj. Only the Q bias stays in-kernel.
"""

import sys

sys.path.insert(0, "/opt/trn_rl_repo")

import numpy as np
import ml_dtypes

BF16 = ml_dtypes.bfloat16
F8 = ml_dtypes.float8_e4m3fn

N_CORES = 8
B, T, C = 2, 2048, 1024
H, D = 16, 64
HPC = 4                       # heads per core
NPAIR = 2                     # head pairs per core
F = HPC * D                   # local feature width = 256
TT = T                        # tokens per core = 2048
TCH = 512                     # token chunk (moving-operand width)
NCH = TT // TCH               # 4 token chunks
KB = 128                      # kv block size
NBLK = TT // KB               # 16 kv blocks

_COMPILED = {}


def _build():
    import concourse.bass as bass
    import concourse.mybir as mybir
    import concourse.tile as tile
    from concourse import bacc

    f32, bf16 = mybir.dt.float32, mybir.dt.bfloat16
    f8 = mybir.dt.float8e4
    Exp = mybir.ActivationFunctionType.Exp
    DR = mybir.MatmulPerfMode.DoubleRow

    nc = bacc.Bacc("TRN2", target_bir_lowering=False, debug=False,
                   num_devices=N_CORES)

    # DRAM layouts mirror the SBUF tiles (partition-major) so each logical
    # block moves with ONE dma_start.
    xt = nc.dram_tensor("xt", [128, C // 128, TT], bf16, kind="ExternalInput")
    x8 = nc.dram_tensor("x8", [128, C // 256, 2, TT], f8,
                        kind="ExternalInput")
    w8 = nc.dram_tensor("w8", [128, 4, 4, 2, 128], f8, kind="ExternalInput")
    wv = nc.dram_tensor("wv", [128, C // 128, 2 * 128], bf16,
                        kind="ExternalInput")
    bq2 = nc.dram_tensor("bq2", [128, 2], f32, kind="ExternalInput")
    wp = nc.dram_tensor("wp", [128, 2, C], bf16, kind="ExternalInput")
    # const blob: [tri2 (2*128) | eye (128) | ones (32)]
    cblob = nc.dram_tensor("cblob", [128, 2 * KB + 128 + 32], bf16,
                           kind="ExternalInput")
    out = nc.dram_tensor("out", [TT, C], bf16, kind="ExternalOutput")

    with tile.TileContext(nc) as tc, \
         nc.allow_low_precision(reason="fp8/bf16 matmul pipeline, fp32 psum"):
        with tc.tile_pool(name="const", bufs=1) as cpool, \
             tc.tile_pool(name="seq", bufs=1) as seq, \
             tc.tile_pool(name="work", bufs=4) as work, \
             tc.tile_pool(name="psBig", bufs=2, space="PSUM") as psBig, \
             tc.tile_pool(name="psS", bufs=2, space="PSUM") as psS, \
             tc.tile_pool(name="psY", bufs=2, space="PSUM") as psY:

            # ---- weights first (idle queues) so the first matmuls only
            #      wait on x arrival. Few LARGE transfers: the PE p-state
            #      rewards dense late bursts over early dribbles. The tiny
            #      Q-bias rides right behind w8 (a late bias stalls
            #      attention; bq2 ahead of w8 costs ~2us of fixed latency).
            w8_sb = cpool.tile([128, 4, 4, 2, 128], f8)
            nc.scalar.dma_start(w8_sb[:], w8[:])
            x8_sb = cpool.tile([128, 4, 2, TT], f8)
            nc.sync.dma_start(x8_sb[:, :, :, 0:TCH], x8[:, :, :, 0:TCH])
            b_sb = cpool.tile([128, 2], f32)
            nc.scalar.dma_start(b_sb[:], bq2[:])
            cb_sb = cpool.tile([128, 2 * KB + 128 + 32], bf16)
            nc.scalar.dma_start(cb_sb[:], cblob[:])
            tri_sb = cb_sb[:, 0:2 * KB].rearrange("p (a q) -> p a q", a=2)
            eye_sb = cb_sb[:, 2 * KB:2 * KB + 128]
            ones_sb = cb_sb[:, 2 * KB + 128:]
            wv_sb = cpool.tile([128, 8, 2 * 128], bf16)
            nc.scalar.dma_start(wv_sb[:], wv[:])

            xt_sb = cpool.tile([128, C // 128, TT], bf16)
            nc.gpsimd.dma_start(xt_sb[:, :, 0:TCH], xt[:, :, 0:TCH])
            wp_sb = cpool.tile([128, 2, C], bf16)
            nc.scalar.dma_start(wp_sb[:], wp[:])
            for t in range(1, NCH):
                tsl = slice(t * TCH, (t + 1) * TCH)
                nc.sync.dma_start(x8_sb[:, :, :, tsl], x8[:, :, :, tsl])
                nc.gpsimd.dma_start(xt_sb[:, :, tsl], xt[:, :, tsl])

            # ---- resident sequence tensors, per (pair, 512-token chunk) ----
            qt_t = [[seq.tile([128, TCH], bf16, tag=f"qt{p}{t}",
                              name=f"qt{p}{t}") for t in range(NCH)]
                    for p in range(NPAIR)]
            kt_t = [[seq.tile([128, TCH], bf16, tag=f"kt{p}{t}",
                              name=f"kt{p}{t}") for t in range(NCH)]
                    for p in range(NPAIR)]
            yt_t = [[seq.tile([128, TCH], bf16, tag=f"yt{p}{t}",
                              name=f"yt{p}{t}") for t in range(NCH)]
                    for p in range(NPAIR)]
            # v65[p][:, i, 0:65] = [V_head0 | 1], [:, i, 65:130] = [V_head1 | 1]
            v65 = []
            for p in range(NPAIR):
                v = seq.tile([128, NBLK, 130], bf16, name=f"v65{p}")
                nc.vector.tensor_copy(
                    v[:, :, 64::65],
                    ones_sb[:, :].rearrange("p (a b) -> p a b", b=2))
                v65.append(v)

            # ---- pre-zeroed diagonal P tiles ([128, 1024], halves=heads) ----
            p_diag = {}
            for r in (1, 2, 3):
                for j in (0, 1):
                    pt = seq.tile([128, 2 * TCH], bf16, name=f"pdiag{r}_{j}")
                    pt3 = pt[:].rearrange("p (a q) -> p a q", a=2)
                    nc.vector.memset(pt3[:, :, 0:128 * r].bitcast(f32), 0.0)
                    p_diag[(r, j)] = pt

            # per-(bq,pair) denominator tiles [1, 2*TCH] fp32 (halves=heads)
            den_tiles = {}

            def qkv_qk_gen(t):
                """Q,K projection for one 512-token chunk (fp8 DoubleRow,
                256-deep slabs)."""
                tsl = slice(t * TCH, (t + 1) * TCH)
                for s in range(4):
                    part, pair = s // 2, s % 2
                    ps = psBig.tile([128, TCH], f32, tag="big",
                                    name=f"pqk{t}_{s}")
                    for sl in range(4):
                        nc.tensor.matmul(
                            ps[:], w8_sb[:, sl, s], x8_sb[:, sl, :, tsl],
                            start=(sl == 0), stop=(sl == 3), perf_mode=DR)
                        if sl == 1:
                            yield
                    if part == 0:
                        # only Q keeps a bias (K's drops under softmax
                        # shift-invariance, V's folds into b_proj); t0 on
                        # the then-idle ACT engine
                        if t == 0:
                            nc.scalar.add(qt_t[pair][t][:], ps[:],
                                          b_sb[:, s:s + 1])
                        else:
                            nc.vector.tensor_scalar_add(qt_t[pair][t][:],
                                                        ps[:],
                                                        b_sb[:, s:s + 1])
                    else:
                        nc.vector.tensor_copy(kt_t[pair][t][:], ps[:])
                    yield

            def qkv_v_gen(t):
                """V projection + transpose into v65 for one chunk."""
                tsl = slice(t * TCH, (t + 1) * TCH)
                # V: bf16, 128-deep blocks
                vt_tmp = [None, None]
                for pair in range(NPAIR):
                    ps = psBig.tile([128, TCH], f32, tag="big",
                                    name=f"pv{t}_{pair}")
                    for cb in range(8):
                        nc.tensor.matmul(
                            ps[:], wv_sb[:, cb, pair * 128:(pair + 1) * 128],
                            xt_sb[:, cb, tsl],
                            start=(cb == 0), stop=(cb == 7))
                        if cb == 3:
                            yield
                    vt_tmp[pair] = work.tile([128, TCH], bf16, tag="vt",
                                             bufs=2, name=f"vt{t}_{pair}")
                    nc.vector.tensor_copy(vt_tmp[pair][:], ps[:])
                    yield
                for pair in range(NPAIR):
                    ptr = psS.tile([128, TCH], bf16, tag="s",
                                   name=f"ptr{t}_{pair}")
                    for i in range(4):
                        nc.tensor.transpose(ptr[:, i * 128:(i + 1) * 128],
                                            vt_tmp[pair][:, i * 128:(i + 1) * 128],
                                            eye_sb[:])
                        if i == 1:
                            yield
                    # single strided copy: [kv, blk, head, d] <- [kv, blk*d]
                    t4 = t * 4
                    nc.vector.tensor_copy(
                        v65[pair][:, t4:t4 + 4, :].rearrange(
                            "p a (h c) -> p a h c", c=65)[:, :, :, 0:64],
                        ptr[:].rearrange("p (a h c) -> p a h c", a=4, h=2))
                    yield

            def norm_pair_gen(bq, pair, fused_ps=None):
                """Softmax normalization for one head pair of a chunk.
                With fused_ps (tail path) the psum->sbuf copy and the
                normalize multiply collapse into one scalar_tensor_tensor,
                shortening the critical tail chain."""
                rec = work.tile([1, 2 * TCH], f32, tag="rec", bufs=4,
                                name=f"rec{bq}{pair}")
                if fused_ps is not None:
                    # tail path: per-head pipeline, reciprocal -> PE ones-row
                    # broadcast (PE is idle; gpsimd serializes) -> fused
                    # psum-normalize-copy
                    for hh in range(2):
                        hs = hh * 64
                        hsl = slice(hh * TCH, (hh + 1) * TCH)
                        nc.vector.reciprocal_approx_fast(
                            rec[:, hsl], den_tiles[(bq, pair)][:, hsl])
                        bcast = work.tile([128, TCH], f32, tag="bcast",
                                          bufs=4, name=f"bcf{pair}{hh}")
                        nc.gpsimd.partition_broadcast(bcast[:], rec[:, hsl])
                        nc.vector.scalar_tensor_tensor(
                            yt_t[pair][bq][hs:hs + 64, :],
                            fused_ps[hh][0:64, :], 1.0,
                            bcast[0:64, :],
                            op0=mybir.AluOpType.mult,
                            op1=mybir.AluOpType.mult)
                        yield
                    return
                nc.vector.reciprocal_approx_fast(rec[:],
                                                 den_tiles[(bq, pair)][:])
                yield
                for hh in range(2):
                    hs = hh * 64
                    bcast = work.tile([128, TCH], f32, tag="bcast", bufs=4,
                                      name=f"bcast{bq}{pair}{hh}")
                    nc.gpsimd.partition_broadcast(
                        bcast[:], rec[:, hh * TCH:(hh + 1) * TCH])
                    nc.vector.tensor_mul(yt_t[pair][bq][hs:hs + 64, :],
                                         yt_t[pair][bq][hs:hs + 64, :],
                                         bcast[hs:hs + 64, :])
                    yield

            def proj_gen(bq, tail=False):
                """c_proj for one 512-token chunk. At the tail, psS is free
                (exp done) - borrow it so six psum regions are in flight and
                the psum->sbuf copies leave the matmul critical path."""
                ps_wide = {}
                if tail:
                    for ic in (0, 1):
                        ps_wide[ic] = psS.tile([128, 2 * TCH], f32, tag="s",
                                               name=f"pjw{ic}")
                for ic in range(4):
                    tt_i = bq * 4 + ic
                    for cc in range(2):
                        if ic in ps_wide:
                            pj = ps_wide[ic][:, cc * TCH:(cc + 1) * TCH]
                        else:
                            pj = psBig.tile([128, TCH], f32, tag="big",
                                            name=f"pj{tt_i}_{cc}")[:]
                        for pair in range(NPAIR):
                            nc.tensor.matmul(
                                pj,
                                yt_t[pair][bq][:, ic * 128:(ic + 1) * 128],
                                wp_sb[:, pair, cc * TCH:(cc + 1) * TCH],
                                start=(pair == 0), stop=(pair == 1))
                        ost = work.tile([128, TCH], bf16, tag="ost", bufs=8,
                                        name=f"ost{tt_i}_{cc}")
                        if tail and (ic + cc) % 2 == 0:
                            nc.scalar.copy(ost[:], pj)
                        else:
                            nc.vector.tensor_copy(ost[:], pj)
                        # steady-state stores ride the sync queue; at the
                        # tail the queues split the last 1MB with gpsimd
                        # first (so its end-of-program drain overlaps proj)
                        # and scalar last (after its ost copies)
                        if tail:
                            qs = (nc.gpsimd, nc.gpsimd, nc.sync, nc.sync,
                                  nc.sync, nc.sync, nc.scalar, nc.scalar)
                            eng = qs[ic * 2 + cc]
                        else:
                            eng = nc.sync
                        eng.dma_start(
                            out[tt_i * 128:(tt_i + 1) * 128,
                                cc * TCH:(cc + 1) * TCH], ost[:])
                        yield

            class Filler:
                """Paces background generators. Tile deps only attach to
                already-emitted producers, so a consumer must never be
                emitted before its producer: drain_through(k) force-emits
                every generator keyed <= k before its consumers go out."""

                def __init__(self):
                    self.gens = []

                def add(self, g, key=None):
                    self.gens.append((key, g))

                def step(self):
                    while self.gens:
                        try:
                            next(self.gens[0][1])
                            return
                        except StopIteration:
                            self.gens.pop(0)

                def drain_through(self, key):
                    while self.gens and self.gens[0][0] is not None \
                            and self.gens[0][0] <= key:
                        k, g = self.gens.pop(0)
                        for _ in g:
                            pass

                def drain(self):
                    while self.gens:
                        for _ in self.gens.pop(0)[1]:
                            pass

            def attn_pair(pair, bq, bk, use_idx):
                """S for a head pair into one [128,1024] psum tile + one exp.
                The two heads' S matmuls run concurrently on PE row groups
                0:64 / 64:128. exp applies the 1/sqrt(d) softmax scale."""
                kchunk = bk // 4
                kcol = (bk % 4) * 128
                s_ps = psS.tile([128, 2 * TCH], f32, tag="s",
                                name=f"s{pair}{bq}{bk}")
                r = bk - 4 * bq
                trim = 128 * r if r > 0 else 0
                for hh in range(2):
                    hs = hh * 64
                    nc.tensor.matmul(
                        s_ps[:, hh * TCH + trim:(hh + 1) * TCH],
                        kt_t[pair][kchunk][hs:hs + 64, kcol:kcol + 128],
                        qt_t[pair][bq][hs:hs + 64, trim:],
                        start=True, stop=True)
                if r < 0:
                    p_t = work.tile([128, 2 * TCH], bf16, tag="p", bufs=6,
                                    name=f"p{pair}{bq}{bk}")
                    nc.scalar.activation(p_t[:], s_ps[:], Exp, scale=0.125)
                    return p_t
                if r == 0:
                    p_t = work.tile([128, 2 * TCH], bf16, tag="p", bufs=6,
                                    name=f"p{pair}{bq}{bk}")
                    nc.scalar.activation(p_t[:], s_ps[:], Exp, scale=0.125)
                else:
                    p_t = p_diag[(r, use_idx % 2)]
                    s3 = s_ps[:].rearrange("p (a q) -> p a q", a=2)
                    p3 = p_t[:].rearrange("p (a q) -> p a q", a=2)
                    nc.scalar.activation(p3[:, :, 128 * r:],
                                         s3[:, :, 128 * r:], Exp, scale=0.125)
                # one strided mask multiply covering both heads
                p3m = p_t[:].rearrange("p (a q) -> p a q", a=2)
                nc.vector.tensor_mul(p3m[:, :, 128 * r:128 * r + 128],
                                     p3m[:, :, 128 * r:128 * r + 128],
                                     tri_sb[:])
                return p_t

            def attention_pair_chunk(pair, bq, fl, fuse=False, pre=None):
                nsteps = 1
                nblk = 4 * bq + 4
                yt_ps = [psY.tile([65, TCH], f32, tag="yt",
                                  name=f"ytps{pair}{bq}{hh}")
                         for hh in range(2)]
                pend = None

                def emit_pv(bk, p_t, stop):
                    r = bk - 4 * bq
                    trim = 128 * r if r > 0 else 0
                    for hh in range(2):
                        nc.tensor.matmul(
                            yt_ps[hh][:, trim:],
                            v65[pair][:, bk, 65 * hh:65 * hh + 65],
                            p_t[:, hh * TCH + trim:(hh + 1) * TCH],
                            start=(bk == 0), stop=stop)

                for bk in range(nblk):
                    if pre is not None:
                        p_t = pre[(pair, bk)]
                    else:
                        p_t = attn_pair(pair, bq, bk, bq + pair)
                    if pend is not None:
                        emit_pv(pend[0], pend[1], stop=False)
                    pend = (bk, p_t)
                    for _ in range(nsteps):
                        fl.step()
                emit_pv(pend[0], pend[1], stop=True)
                den = work.tile([1, 2 * TCH], f32, tag="den", bufs=10,
                                name=f"den{bq}{pair}")
                for hh in range(2):
                    hs = hh * 64
                    if fuse:
                        # tail: ACT is idle, keep DVE free for the recips
                        nc.scalar.copy(den[:, hh * TCH:(hh + 1) * TCH],
                                       yt_ps[hh][64:65, :])
                    else:
                        nc.vector.tensor_copy(
                            den[:, hh * TCH:(hh + 1) * TCH],
                            yt_ps[hh][64:65, :])
                        nc.vector.tensor_copy(yt_t[pair][bq][hs:hs + 64, :],
                                              yt_ps[hh][0:64, :])
                den_tiles[(bq, pair)] = den
                fl.step()
                return yt_ps

            def chain(*gens):
                for g in gens:
                    yield from g

            # ---- schedule ----
            # chunk-0 Q,K first; then bq0's S+exp (they only need q,k and
            # run while xt/V still stream in); V emits after so the PE
            # in-order queue isn't blocked by transposes waiting on xt
            for _ in qkv_qk_gen(0):
                pass
            pre0 = {}
            for pair in range(NPAIR):
                for bk in range(4):
                    pre0[(pair, bk)] = attn_pair(pair, 0, bk, pair)
            for _ in qkv_v_gen(0):
                pass
            fl = Filler()
            for t in range(1, NCH):
                fl.add(chain(qkv_qk_gen(t), qkv_v_gen(t)), key=t)
            pending_np = []
            last = NCH - 1
            for bq in range(NCH):
                fl.drain_through(bq)
                if bq == last:
                    # make all pending norm+proj work available as filler
                    # so only the final chunk's chain remains at the tail
                    while pending_np:
                        fl.add(pending_np.pop(0))
                for pair in range(NPAIR):
                    fuse = (bq == last and pair == NPAIR - 1)
                    yt_ps = attention_pair_chunk(
                        pair, bq, fl, fuse=fuse,
                        pre=pre0 if bq == 0 else None)
                    if bq == last:
                        # normalize each pair while the next one (or the
                        # tail) is still streaming
                        fl.add(norm_pair_gen(bq, pair,
                                             fused_ps=yt_ps if fuse
                                             else None))
                if bq != last:
                    if pending_np:
                        fl.add(pending_np.pop(0))
                    pending_np.append(chain(norm_pair_gen(bq, 0),
                                            norm_pair_gen(bq, 1),
                                            proj_gen(bq)))
                else:
                    fl.add(proj_gen(bq, tail=True))
            fl.drain()
    nc.compile()
    return nc


def _get_nc():
    if "nc" not in _COMPILED:
        _COMPILED["nc"] = _build()
    return _COMPILED["nc"]


def _prep_in_maps(x, w_attn, b_attn, w_proj):
    x = np.asarray(x, np.float32)
    w_attn = np.asarray(w_attn, np.float32)
    b_attn = np.asarray(b_attn, np.float32)
    w_proj = np.asarray(w_proj, np.float32)

    tri = np.triu(np.ones((KB, KB), np.float32))  # keep where q >= kv
    eye = np.eye(128, dtype=np.float32)
    ones = np.ones((128, 32), np.float32)
    cblob = np.ascontiguousarray(np.concatenate(
        [np.stack([tri, tri], axis=1).reshape(128, 2 * KB), eye, ones],
        axis=1)).astype(BF16)

    # xt[p, cb, t] = x^T[cb*128 + p, t]
    xts = [np.ascontiguousarray(
        x[b].T.reshape(8, 128, T).transpose(1, 0, 2)).astype(BF16)
        for b in range(B)]
    # x8[p, sl, i, t] = x^T[sl*256 + i*128 + p, t]
    x8s = [np.ascontiguousarray(
        x[b].T.reshape(4, 2, 128, T).transpose(2, 0, 1, 3)).astype(F8)
        for b in range(B)]

    in_maps = []
    for c in range(N_CORES):
        b, hg = c // 4, c % 4
        f0 = hg * F
        # w8[p, sl, s, i, cc] = w[sl*256 + i*128 + p, col(s, cc)], unscaled
        w8a = np.zeros((128, 4, 4, 2, 128), np.float32)
        slices_bq = []
        for s in range(4):
            part, pair = s // 2, s % 2
            lo = part * C + f0 + pair * 128
            wsl = w_attn[:, lo:lo + 128]          # [C, 128]
            if part == 0:
                slices_bq.append(b_attn[lo:lo + 128])
            w8a[:, :, s, :, :] = wsl.reshape(4, 2, 128, 128).transpose(
                2, 0, 1, 3)
        w8a = np.ascontiguousarray(w8a).astype(F8)
        bq2 = np.ascontiguousarray(np.stack(slices_bq, axis=1))
        # wv[p, cb, pair*128+cc] = w_v[cb*128 + p, f0 + pair*128 + cc]
        wva = np.ascontiguousarray(
            w_attn[:, 2 * C + f0:2 * C + f0 + F].reshape(
                8, 128, 256).transpose(1, 0, 2)).astype(BF16)
        # wp[p, pair, cc] = w_proj[f0 + pair*128 + p, cc]
        wpc = np.ascontiguousarray(
            w_proj[f0:f0 + F, :].reshape(2, 128, C).transpose(
                1, 0, 2)).astype(BF16)
        in_maps.append({
            "xt": xts[b], "x8": x8s[b], "w8": w8a, "wv": wva,
            "bq2": bq2, "wp": wpc, "cblob": cblob,
        })
    return in_maps


def _run(inputs, trace=False):
    from concourse.bass_utils import run_bass_kernel_spmd

    nc = _get_nc()
    in_maps = _prep_in_maps(inputs["x"], inputs["w_attn"], inputs["b_attn"],
                            inputs["w_proj"])
    res = run_bass_kernel_spmd(nc, in_maps, list(range(N_CORES)), trace=trace)
    b_attn = np.asarray(inputs["b_attn"], np.float32)
    w_proj = np.asarray(inputs["w_proj"], np.float32)
    # V bias passes through softmax (rows sum to 1) -> fold into b_proj
    b_eff = (np.asarray(inputs["b_proj"], np.float64)
             + b_attn[2 * C:].astype(np.float64) @ w_proj.astype(np.float64))
    y = np.zeros((B, TT, C), np.float32)
    for b in range(B):
        acc = np.zeros((TT, C), np.float64)
        for hg in range(4):
            acc += np.asarray(res.results[b * 4 + hg]["out"], np.float64)
        y[b] = (acc + b_eff).astype(np.float32)
    return y, res


def kernel(**inputs):
    y, _ = _run(inputs, trace=False)
    return y
